# revision 64
# baseline (speedup 1.0000x reference)
"""Trainium2 Bass kernel for ChannelSqueezeSpatialAttention.

Reference computation (shapes hardcoded):
  xq  [4, 256, 64, 64], xkv [4, 256, 32, 32]
  wq/wk/wv [256, 256], emb_q/emb_k [17, 64, 3, 7, 7]
  q = wq @ xq (1x1 conv), k = wk @ xkv, v = wv @ xkv
  q_c = conv3d(q, emb_q) over (head, y, x) with kernel (3,7,7) -> 17 ch/head
  k_c = conv3d(k, emb_k)
  sim = softmax(q_c^T k_c / 8), rec = sim @ v  -> [4, 256, 64, 64]

Sharding: 8 cores = 4 batches x 2 head-pairs. Each core computes 2 heads of
one batch. The conv mixes adjacent heads (3-wide along head axis), so each
core computes q/k projections for its pair-relative head slots r0..r3 =
heads (2p-1, 2p, 2p+1, 2p+2); out-of-range slots get zero weight columns
host-side (no halo exchange needed).

Conv mapping: shift-and-accumulate matmuls with M = (ky, cg) = 7*17 = 119
packed output rows. The ky-summation is deferred: partial planes are stored
to a y-padded SBUF tensor with a per-ky y-shift via ONE DMA whose hand-built
access pattern strides both partitions (+17) and bytes (-row) per ky group.
The scores matmul contracts over (ky, cg) with a ky-replicated k_c as the
stationary operand, which completes the convolution for free.

Attention: scores computed transposed S^T[sk, sq] so softmax-exp output E^T
feeds the value matmul directly: rec^T[d|Z, sq] = [v|1]^T E^T. 1/Z via DVE
reciprocal on a transpose-gathered [16,64] tile + K=1 broadcast matmul +
DVE multiply.

Dtypes: everything on the PE in bf16 except the zb broadcast (bf16) and
f32 PSUM accumulation; E in bf16 halves the value-matmul cycles.

DMA queues: xq preload on the scalar queue, out stores on gpsimd, the rest
(consts, plane shifts, conv scatters, z transposes) on sync — avoids
head-of-line blocking on a single queue (each DMA issue costs ~0.6us).
"""

import functools
import numpy as np
import ml_dtypes

import bass_rust
import concourse.bass as bass
import concourse.tile as tile
import concourse.mybir as mybir
from concourse import bacc
from concourse.bass_utils import run_bass_kernel_spmd

F32 = mybir.dt.float32
F32R = mybir.dt.float32r
BF16 = mybir.dt.bfloat16

B = 4
NH = 4
D = 64            # head dim
CG = 17           # squeezed channels
K7 = 7            # spatial kernel
HQ = 64           # q image h=w
HK = 32           # k image h=w
SQ = HQ * HQ      # 4096
SK = HK * HK      # 1024
MC = K7 * CG      # 119 conv output rows (ky, cg)
QP = HQ + 6       # 70: x-padded q row width
KP = HK + 6       # 38: x-padded k row width
SCALE = D ** -0.5

QCH = 8           # q spatial chunks (8 y-rows each)
KCH = 2           # k spatial chunks (16 y-rows each)
QROWS = HQ // QCH  # 8
KROWS = HK // KCH  # 16
NSLAB = SQ // 512  # 8 sq slabs per head
FSQ = (HQ + 6) * HQ   # 4480: y-padded pq free size
FSK = (HK + 6) * HK   # 1216: y-padded pk free size
BODYQ = 3 * HQ        # 192: body column offset in pq
BODYK = 3 * HK        # 96

AF = mybir.ActivationFunctionType


def _build_program():
    nc = bacc.Bacc()

    xq = nc.dram_tensor("xq", [256, SQ], BF16, kind="ExternalInput")
    xkv = nc.dram_tensor("xkv", [256, SK], BF16, kind="ExternalInput")
    wqT = nc.dram_tensor("wqT", [256, 512], BF16, kind="ExternalInput")
    wkT = nc.dram_tensor("wkT", [256, 512], BF16, kind="ExternalInput")
    wvT = nc.dram_tensor("wvT", [256, 128], BF16, kind="ExternalInput")
    wcq = nc.dram_tensor("wcq", [128, 7, MC], BF16, kind="ExternalInput")
    wcq2 = nc.dram_tensor("wcq2", [128, 4, MC], BF16, kind="ExternalInput")
    wck = nc.dram_tensor("wck", [128, 7, MC], BF16, kind="ExternalInput")
    wck2 = nc.dram_tensor("wck2", [128, 4, MC], BF16, kind="ExternalInput")
    repl = nc.dram_tensor("repl", [MC, MC], BF16, kind="ExternalInput")
    out = nc.dram_tensor("out", [128, SQ], BF16, kind="ExternalOutput")

    with tile.TileContext(nc) as tc:
        _emit(nc, tc, xq, xkv, wqT, wkT, wvT, wcq, wcq2, wck, wck2, repl, out)
    nc.compile()
    return nc


def _shift_ap(dst_tile, y0, w, fs, cnt):
    """AP over a flat DRAM stage writing [119, cnt] planes with a per-ky
    y-shift: plane g (rows g*17..) lands cnt cols at (y0+6-g)*w. Mixed
    row/column strides are only legal on the DRAM side, hence the bounce."""
    v = dst_tile[:, (y0 + 6) * w:(y0 + 6) * w + cnt].copy()
    v.ap = bass_rust.VecI64Pair([[CG * fs - w, K7], [fs, CG], [1, cnt]])
    return v


def _emit(nc, tc, xq, xkv, wqT, wkT, wvT, wcq, wcq2, wck, wck2, repl, out):
    import contextlib
    ctx = contextlib.ExitStack()
    with ctx:
        consts = ctx.enter_context(tc.tile_pool(name="consts", bufs=1))
        stg = ctx.enter_context(tc.tile_pool(name="stg", bufs=3))
        dstg = ctx.enter_context(tc.tile_pool(name="dstg", bufs=2, space="DRAM"))
        pqp = ctx.enter_context(tc.tile_pool(name="pqp", bufs=2))
        pkp = ctx.enter_context(tc.tile_pool(name="pkp", bufs=2))
        k2p = ctx.enter_context(tc.tile_pool(name="k2p", bufs=2))
        ep = ctx.enter_context(tc.tile_pool(name="ep", bufs=2))
        rp = ctx.enter_context(tc.tile_pool(name="rp", bufs=2))
        zp = ctx.enter_context(tc.tile_pool(name="zp", bufs=2))
        op = ctx.enter_context(tc.tile_pool(name="op", bufs=3))
        ps_mm = ctx.enter_context(tc.tile_pool(name="ps_mm", bufs=2, space="PSUM"))
        ps_sc = ctx.enter_context(tc.tile_pool(name="ps_sc", bufs=2, space="PSUM"))
        ps_rec = ctx.enter_context(tc.tile_pool(name="ps_rec", bufs=2, space="PSUM"))

        # ---- xq preload on the scalar DMA queue (16 DMAs -> 16 engines) ----
        xq_sb = consts.tile([128, 2, SQ], BF16)
        for ct in range(2):
            for cb in range(8):
                nc.scalar.dma_start(
                    xq_sb[:, ct, cb * 512:(cb + 1) * 512],
                    xq[ct * 128:(ct + 1) * 128, cb * 512:(cb + 1) * 512])

        # ---- constant loads (sync queue), in first-use order: the v/k
        # projections only need xkv + small weights, so those go first and
        # the PE starts within a few us ----
        xkv_sb = consts.tile([128, 2, SK], BF16)
        for ct in range(2):
            for cb in range(2):
                nc.sync.dma_start(
                    xkv_sb[:, ct, cb * 512:(cb + 1) * 512],
                    xkv[ct * 128:(ct + 1) * 128, cb * 512:(cb + 1) * 512])
        wkT_sb = consts.tile([128, 2, 512], BF16)
        for ct in range(2):
            nc.sync.dma_start(wkT_sb[:, ct, :],
                              wkT[ct * 128:(ct + 1) * 128, :])
        wvT_sb = consts.tile([128, 2, 128], BF16)
        nc.sync.dma_start(wvT_sb, wvT.rearrange("(t p) m -> p t m", t=2))
        wck_sb = consts.tile([128, 7, MC], BF16)
        wck2_sb = consts.tile([128, 4, MC], BF16)
        nc.sync.dma_start(wck_sb, wck[:])
        nc.sync.dma_start(wck2_sb, wck2[:])
        wqT_sb = consts.tile([128, 2, 512], BF16)
        for ct in range(2):
            nc.sync.dma_start(wqT_sb[:, ct, :],
                              wqT[ct * 128:(ct + 1) * 128, :])
        repl_sb = consts.tile([MC, MC], BF16)
        nc.sync.dma_start(repl_sb, repl[:])
        wcq_sb = consts.tile([128, 7, MC], BF16)
        wcq2_sb = consts.tile([128, 4, MC], BF16)
        nc.sync.dma_start(wcq_sb, wcq[:])
        nc.sync.dma_start(wcq2_sb, wcq2[:])
        ones0 = consts.tile([1, 64], BF16)
        nc.vector.memset(ones0[:], 1.0)
        zeros_sb = consts.tile([CG, 192], BF16)
        nc.vector.memset(zeros_sb[:], 0.0)

        def zero_stage_garbage(ds, w, rows):
            """Zero the never-written body-edge regions of a shifted DRAM
            stage: plane g only covers body cols from (3-g)*w (low) and up
            to (rows+3-g+3)*w (high)."""
            body0, body1 = 3 * w, (3 + rows) * w
            for g in range(3):
                gw = (3 - g) * w
                nc.scalar.dma_start(
                    ds[g * CG:(g + 1) * CG, body0:body0 + gw],
                    zeros_sb[:, 0:gw])
            for g in range(4, 7):
                gw = (g - 3) * w
                nc.scalar.dma_start(
                    ds[g * CG:(g + 1) * CG, body1 - gw:body1],
                    zeros_sb[:, 0:gw])

        # q/k plane tensors: qA=(r0,r1), qB=(r1,r2) for the main conv
        # group; qD0=(r2|r2 x-shifted), qD1=(r3|r3 x-shifted) so the
        # 64-wide third slot contracts two kx taps per matmul (the shifted
        # half holds tap kx+1 at the same padded column window).
        qA = consts.tile([128, HQ * QP], BF16)
        qB = consts.tile([128, HQ * QP], BF16)
        qD0 = consts.tile([128, HQ * QP], BF16)
        qD1 = consts.tile([128, HQ * QP], BF16)
        kA = consts.tile([128, HK * KP], BF16)
        kB = consts.tile([128, HK * KP], BF16)
        kD0 = consts.tile([128, HK * KP], BF16)
        kD1 = consts.tile([128, HK * KP], BF16)

        # x-border zeros; shifted halves sit one column left
        for t in (qA, qB):
            v = t[:].rearrange("p (r c) -> p r c", c=QP)
            nc.vector.memset(v[:, :, 0:3], 0.0)
            nc.vector.memset(v[:, :, QP - 3:QP], 0.0)
        for t in (qD0, qD1):
            v = t[:].rearrange("p (r c) -> p r c", c=QP)
            nc.vector.memset(v[0:64, :, 0:3], 0.0)
            nc.vector.memset(v[0:64, :, QP - 3:QP], 0.0)
            nc.vector.memset(v[64:128, :, 0:2], 0.0)
            nc.vector.memset(v[64:128, :, QP - 4:QP], 0.0)
        for t in (kA, kB):
            v = t[:].rearrange("p (r c) -> p r c", c=KP)
            nc.vector.memset(v[:, :, 0:3], 0.0)
            nc.vector.memset(v[:, :, KP - 3:KP], 0.0)
        for t in (kD0, kD1):
            v = t[:].rearrange("p (r c) -> p r c", c=KP)
            nc.vector.memset(v[0:64, :, 0:3], 0.0)
            nc.vector.memset(v[0:64, :, KP - 3:KP], 0.0)
            nc.vector.memset(v[64:128, :, 0:2], 0.0)
            nc.vector.memset(v[64:128, :, KP - 4:KP], 0.0)

        # v^T projection: out[sk_blk, (h0 d | h1 d)]
        v_sb = consts.tile([128, 8, 130], BF16)
        nc.vector.memset(v_sb[:, :, 64:65], 1.0)
        nc.vector.memset(v_sb[:, :, 129:130], 1.0)

        def v_proj():
            for t in range(8):
                acc = ps_mm.tile([128, 512], F32, tag="mm", name="accv")
                for ct in range(2):
                    nc.tensor.matmul(
                        acc[:, 0:128],
                        xkv_sb[:, ct, t * 128:(t + 1) * 128],
                        wvT_sb[:, ct, :],
                        start=(ct == 0), stop=(ct == 1),
                    )
                nc.vector.tensor_copy(v_sb[:, t, 0:64], acc[:, 0:64])
                nc.vector.tensor_copy(v_sb[:, t, 65:129], acc[:, 64:128])

        # k projection -> kA/kB/kD0/kD1 padded planes (bf16); weight
        # column groups host-built as (r0,r1),(r1,r2),(r2,r2),(r3,r3).
        def k_proj():
            for ch in range(2):
                for mt, full in enumerate((kA, kB, kD0, kD1)):
                    acc = ps_mm.tile([128, 512], F32, tag="mm", name="acckp")
                    for ct in range(2):
                        nc.tensor.matmul(
                            acc[:],
                            wkT_sb[:, ct, mt * 128:(mt + 1) * 128],
                            xkv_sb[:, ct, ch * 512:(ch + 1) * 512],
                            start=(ct == 0), stop=(ct == 1),
                        )
                    rows = 512 // HK  # 16
                    y0 = ch * rows
                    dstf = full[:].rearrange("p (r c) -> p r c", c=KP)
                    accv = acc[:].rearrange("p (r c) -> p r c", c=HK)
                    if mt < 2:
                        nc.vector.tensor_copy(
                            dstf[:, y0:y0 + rows, 3:3 + HK], accv)
                    else:
                        # halves land at different x offsets; run on the
                        # idle scalar engine to keep DVE off the critical path
                        nc.scalar.copy(
                            dstf[0:64, y0:y0 + rows, 3:3 + HK], accv[0:64])
                        nc.scalar.copy(
                            dstf[64:128, y0:y0 + rows, 2:2 + HK], accv[64:128])

        # q projection chunk -> qA/qB/qD0/qD1 (bf16)
        def q_proj_chunk(ch):
            for mt, full in enumerate((qA, qB, qD0, qD1)):
                acc = ps_mm.tile([128, 512], F32, tag="mm", name="accqp")
                for ct in range(2):
                    nc.tensor.matmul(
                        acc[:],
                        wqT_sb[:, ct, mt * 128:(mt + 1) * 128],
                        xq_sb[:, ct, ch * 512:(ch + 1) * 512],
                        start=(ct == 0), stop=(ct == 1),
                    )
                rows = 512 // HQ  # 8
                y0 = ch * rows
                dstf = full[:].rearrange("p (r c) -> p r c", c=QP)
                accv = acc[:].rearrange("p (r c) -> p r c", c=HQ)
                if mt < 2:
                    nc.vector.tensor_copy(
                        dstf[:, y0:y0 + rows, 3:3 + HQ], accv)
                else:
                    nc.scalar.copy(
                        dstf[0:64, y0:y0 + rows, 3:3 + HQ], accv[0:64])
                    nc.scalar.copy(
                        dstf[64:128, y0:y0 + rows, 2:2 + HQ], accv[64:128])

        # ---- per-head stage state ----
        pq_t = [None, None]
        pk_t = [None, None]
        k2_t = [None, None]
        rec_sb_t = [None, None]

        dsk_t = [None, None]
        dsq_t = [None, None]

        def conv_k_chunk(h, ch):
            k128 = (kA, kB)[h]
            if ch == 0:
                dsk_t[h] = dstg.tile([MC, FSK], BF16, tag="dsk", name="dsk")
                zero_stage_garbage(dsk_t[h], HK, HK)
            acc = ps_mm.tile([MC, 512], F32, tag="mm", name="acck")
            y0 = ch * KROWS
            for kx in range(K7):
                r1 = k128[:].rearrange("p (r c) -> p r c", c=KP)[
                    :, y0:y0 + KROWS, kx:kx + HK]
                nc.tensor.matmul(acc[:], wck_sb[:, kx, :], r1,
                                 start=(kx == 0), stop=False)
            kDh = (kD0, kD1)[h]
            for j in range(4):
                r2 = kDh[:].rearrange("p (r c) -> p r c", c=KP)[
                    :, y0:y0 + KROWS, 2 * j:2 * j + HK]
                nc.tensor.matmul(acc[:], wck2_sb[:, j, :], r2,
                                 start=False, stop=(j == 3))
            st = stg.tile([MC, 512], BF16, tag="stg", name="stk")
            nc.vector.tensor_copy(st[:], acc[:])
            nc.sync.dma_start(_shift_ap(dsk_t[h], y0, HK, FSK, KROWS * HK),
                              st[:])

        def load_pk(h):
            pk = pkp.tile([MC, FSK], BF16, tag="pk", name="pk")
            pk_t[h] = pk
            for i in range(2):
                c0 = BODYK + i * 512
                nc.scalar.dma_start(pk[:, c0:c0 + 512],
                                    dsk_t[h][:, c0:c0 + 512])

        def k2_build(h):
            k2 = k2p.tile([MC, SK], BF16, tag="k2", name="k2")
            k2_t[h] = k2
            for ch in range(2):
                acc = ps_mm.tile([MC, 512], F32, tag="mm", name="acc2")
                nc.tensor.matmul(
                    acc[:], repl_sb[:],
                    pk_t[h][:, BODYK + ch * 512:BODYK + (ch + 1) * 512],
                    start=True, stop=True)
                nc.vector.tensor_copy(k2[:, ch * 512:(ch + 1) * 512], acc[:])

        def conv_q_chunk(h, ch):
            q128 = (qA, qB)[h]
            if ch == 0:
                dsq_t[h] = dstg.tile([MC, FSQ], BF16, tag="dsq", name="dsq")
                zero_stage_garbage(dsq_t[h], HQ, HQ)
            acc = ps_mm.tile([MC, 512], F32, tag="mm", name="accq")
            y0 = ch * QROWS
            for kx in range(K7):
                r1 = q128[:].rearrange("p (r c) -> p r c", c=QP)[
                    :, y0:y0 + QROWS, kx:kx + HQ]
                nc.tensor.matmul(acc[:], wcq_sb[:, kx, :], r1,
                                 start=(kx == 0), stop=False)
            qDh = (qD0, qD1)[h]
            for j in range(4):
                r2 = qDh[:].rearrange("p (r c) -> p r c", c=QP)[
                    :, y0:y0 + QROWS, 2 * j:2 * j + HQ]
                nc.tensor.matmul(acc[:], wcq2_sb[:, j, :], r2,
                                 start=False, stop=(j == 3))
            st = stg.tile([MC, 512], BF16, tag="stg", name="stq")
            nc.vector.tensor_copy(st[:], acc[:])
            nc.sync.dma_start(_shift_ap(dsq_t[h], y0, HQ, FSQ, QROWS * HQ),
                              st[:])

        def load_pq_piece(h, i):
            if i == 0:
                pq_t[h] = pqp.tile([MC, FSQ], BF16, tag="pq", name="pq")
            c0 = BODYQ + i * 512
            nc.scalar.dma_start(pq_t[h][:, c0:c0 + 512],
                                dsq_t[h][:, c0:c0 + 512])

        def slab(h, s):
            if s == 0:
                rec_sb_t[h] = rp.tile([65, SQ], F32, tag="recsb", name="recsb")
            e_sb = ep.tile([128, SQ], BF16, tag="e", name="esb")
            for qq in range(4):
                sc = ps_sc.tile([128, 1024], F32, tag="sc", name="sc")
                for bb in range(2):
                    blk = qq * 2 + bb
                    nc.tensor.matmul(
                        sc[:, bb * 512:(bb + 1) * 512],
                        k2_t[h][:, blk * 128:(blk + 1) * 128],
                        pq_t[h][:, BODYQ + s * 512:BODYQ + (s + 1) * 512],
                        start=True, stop=True,
                    )
                nc.scalar.activation(
                    e_sb[:, qq * 1024:(qq + 1) * 1024], sc[:],
                    AF.Exp, scale=SCALE)
            rec = ps_rec.tile([65, 512], F32, tag="rec", name="rec")
            for t in range(8):
                nc.tensor.matmul(
                    rec[:],
                    v_sb[:, t, h * 65:(h + 1) * 65],
                    e_sb[:, t * 512:(t + 1) * 512],
                    start=(t == 0), stop=(t == 7),
                )
            nc.vector.tensor_copy(
                rec_sb_t[h][:, s * 512:(s + 1) * 512], rec[:])

        def divide_pair(h, p):
            """1/Z for slabs 2p,2p+1: transpose Z to 16 partitions via a
            DRAM bounce (SBUF APs can't cross partitions mid-row), DVE
            reciprocal multi-lane, transpose back."""
            rec_sb = rec_sb_t[h]
            c0 = p * 1024
            zrb = zp.tile([1, 1024], BF16, tag="zrb", name="zrb")
            zd1 = dstg.tile([1, 1024], F32, tag="zd1", name="zd1")
            nc.gpsimd.dma_start(zd1[:], rec_sb[64:65, c0:c0 + 1024])
            zt = zp.tile([16, 64], F32, tag="zt", name="zt")
            nc.gpsimd.dma_start(
                zt[:], zd1[0:1, :].rearrange("p (a b) -> (p a) b", a=16))
            zi = zp.tile([16, 64], F32R, tag="zi", name="zi")
            with nc.allow_low_precision(
                    reason="1/Z feeds bf16 broadcast mm"):
                nc.vector.reciprocal(zi[:], zt[:])
            zib = zp.tile([16, 64], BF16, tag="zib", name="zib")
            nc.vector.tensor_copy(zib[:], zi[:])
            zd2 = dstg.tile([1, 1024], BF16, tag="zd2", name="zd2")
            nc.gpsimd.dma_start(
                zd2[0:1, :].rearrange("p (a b) -> (p a) b", a=16), zib[:])
            nc.gpsimd.dma_start(zrb[:], zd2[:])
            emit_outputs(h, p, zrb)

        def emit_outputs(h, p, zrb):
            rec_sb = rec_sb_t[h]
            for j in range(2):
                s = 2 * p + j
                zb = ps_mm.tile([64, 512], F32, tag="mm", name="zb")
                nc.tensor.matmul(zb[:], ones0[:],
                                 zrb[0:1, j * 512:(j + 1) * 512],
                                 start=True, stop=True)
                ot = op.tile([64, 512], BF16, tag="ot", name="ot")
                with nc.allow_low_precision(reason="bf16 output store"):
                    nc.vector.tensor_mul(
                        ot[:], rec_sb[0:64, s * 512:(s + 1) * 512], zb[:])
                nc.gpsimd.dma_start(
                    out[h * 64:(h + 1) * 64, s * 512:(s + 1) * 512], ot[:])

        def divide_tail():
            """Slabs (1,6),(1,7): ACT ln/exp on the idle scalar engine —
            both lns then one exp so only 2 act-table loads hit the tail."""
            rec_sb = rec_sb_t[1]
            zl = zp.tile([1, 1024], F32, tag="zl", name="zl")
            for j, s in enumerate((6, 7)):
                nc.scalar.activation(zl[0:1, j * 512:(j + 1) * 512],
                                     rec_sb[64:65, s * 512:(s + 1) * 512],
                                     AF.Ln)
            zrb = zp.tile([1, 1024], BF16, tag="zrb", name="zrb")
            nc.scalar.activation(zrb[:], zl[:], AF.Exp, scale=-1.0)
            emit_outputs(1, 3, zrb)

        # ---- schedule ----
        # Phase A: k path (no xq dependency) warms the PE while xq preloads.
        k_proj()
        for ch in range(KCH):
            conv_k_chunk(0, ch)
        for ch in range(KCH):
            conv_k_chunk(1, ch)
        v_proj()
        load_pk(0)
        load_pk(1)
        # Phase B: q projection (hides the pk stage-write/load latency
        # before k2 consumes it), conv(h0) with pq gathers interleaved.
        for ch in range(4):
            q_proj_chunk(ch)
        k2_build(0)
        k2_build(1)
        for ch in range(4, QCH):
            q_proj_chunk(ch)
        for ch in range(QCH):
            conv_q_chunk(0, ch)
            if ch >= 1:
                load_pq_piece(0, ch - 1)
        load_pq_piece(0, QCH - 1)
        # Phase C: slabs(h0) with conv(h1) interleaved, then slabs(h1);
        # 1/Z + output divides trail one slab behind their pair.
        for h in range(2):
            for s in range(NSLAB):
                slab(h, s)
                if h == 0:
                    conv_q_chunk(1, s)
                    if s >= 1:
                        load_pq_piece(1, s - 1)
                k = h * NSLAB + s          # global slab index
                if k >= 2 and k % 2 == 0:
                    p = (k - 2) // 2       # pairs 0..6; 7 handled at tail
                    divide_pair(p // 4, p % 4)
            if h == 0:
                load_pq_piece(1, QCH - 1)
        divide_tail()


@functools.lru_cache(maxsize=1)
def _get_program():
    return _build_program()


def _host_inputs(xq, xkv, wq, wk, wv, emb_q, emb_k):
    """Build the 8 per-core input maps."""
    xq = np.ascontiguousarray(xq, dtype=np.float32)
    xkv = np.ascontiguousarray(xkv, dtype=np.float32)

    def conv_w(emb):
        # emb [cg, d, dnk, ky, kx] -> rows (dnk, d), cols (kx, ky, cg).
        # Second group packs kx-tap pairs: rows 0:64 = tap 2j, 64:128 =
        # tap 2j+1 (tap 7 = zeros), matching the x-shifted qD/kD planes.
        arr = np.transpose(np.asarray(emb, np.float32), (2, 1, 4, 3, 0))
        w128 = np.ascontiguousarray(
            arr[0:2].reshape(128, 7, MC).astype(ml_dtypes.bfloat16))
        w64 = arr[2].reshape(64, 7, MC)
        wp = np.zeros((128, 4, MC), np.float32)
        for j in range(4):
            wp[0:64, j] = w64[:, 2 * j]
            if 2 * j + 1 < K7:
                wp[64:128, j] = w64[:, 2 * j + 1]
        wp = np.ascontiguousarray(wp.astype(ml_dtypes.bfloat16))
        return w128, wp

    wcq_, wcq2_ = conv_w(emb_q)
    wck_, wck2_ = conv_w(emb_k)
    repl_ = np.ascontiguousarray(
        np.tile(np.eye(CG, dtype=np.float32), (K7, K7)).astype(
            ml_dtypes.bfloat16))

    wq = np.asarray(wq, np.float32)
    wk = np.asarray(wk, np.float32)
    wv = np.asarray(wv, np.float32)

    in_maps = []
    for core in range(8):
        b, p = divmod(core, 2)
        slot = [np.zeros((256, 64), np.float32) for _ in range(4)]
        kslot = [np.zeros((256, 64), np.float32) for _ in range(4)]
        for j in range(4):
            head = 2 * p + j - 1
            if 0 <= head < NH:
                slot[j] = wq[head * 64:(head + 1) * 64, :].T
                kslot[j] = wk[head * 64:(head + 1) * 64, :].T
        # projection column groups: (r0,r1) (r1,r2) (r2,r2) (r3,r3)
        wqT_ = np.concatenate(
            [slot[0], slot[1], slot[1], slot[2],
             slot[2], slot[2], slot[3], slot[3]], axis=1)
        wkT_ = np.concatenate(
            [kslot[0], kslot[1], kslot[1], kslot[2],
             kslot[2], kslot[2], kslot[3], kslot[3]], axis=1)
        wvT_ = np.ascontiguousarray(wv[p * 128:(p + 1) * 128, :].T)
        in_maps.append(dict(
            xq=np.ascontiguousarray(
                xq[b].reshape(256, SQ).astype(ml_dtypes.bfloat16)),
            xkv=np.ascontiguousarray(
                xkv[b].reshape(256, SK).astype(ml_dtypes.bfloat16)),
            wqT=np.ascontiguousarray(wqT_.astype(ml_dtypes.bfloat16)),
            wkT=np.ascontiguousarray(wkT_.astype(ml_dtypes.bfloat16)),
            wvT=np.ascontiguousarray(wvT_.astype(ml_dtypes.bfloat16)),
            wcq=wcq_, wcq2=wcq2_, wck=wck_, wck2=wck2_,
            repl=repl_,
        ))
    return in_maps


def _run(inputs, **kw):
    nc = _get_program()
    in_maps = _host_inputs(**inputs)
    res = run_bass_kernel_spmd(nc, in_maps, core_ids=list(range(8)), **kw)
    outp = np.empty((B, 256, HQ, HQ), np.float32)
    for core in range(8):
        b, p = divmod(core, 2)
        outp[b, p * 128:(p + 1) * 128] = \
            res.results[core]["out"].reshape(128, HQ, HQ).astype(np.float32)
    return outp, res


def kernel(xq, xkv, wq, wk, wv, emb_q, emb_k):
    outp, _ = _run(dict(xq=xq, xkv=xkv, wq=wq, wk=wk, wv=wv,
                        emb_q=emb_q, emb_k=emb_k))
    return outp


# revision 66
# speedup vs baseline: 1.2999x; 1.2999x over previous
"""Trainium2 Bass kernel for ChannelSqueezeSpatialAttention.

Reference computation (shapes hardcoded):
  xq  [4, 256, 64, 64], xkv [4, 256, 32, 32]
  wq/wk/wv [256, 256], emb_q/emb_k [17, 64, 3, 7, 7]
  q = wq @ xq (1x1 conv), k = wk @ xkv, v = wv @ xkv
  q_c = conv3d(q, emb_q) over (head, y, x) with kernel (3,7,7) -> 17 ch/head
  k_c = conv3d(k, emb_k)
  sim = softmax(q_c^T k_c / 8), rec = sim @ v  -> [4, 256, 64, 64]

Sharding: 8 cores = 4 batches x 2 head-pairs. Each core computes 2 heads of
one batch. The conv mixes adjacent heads (3-wide along head axis), so each
core computes q/k projections for its pair-relative head slots r0..r3 =
heads (2p-1, 2p, 2p+1, 2p+2); out-of-range slots get zero weight columns
host-side (no halo exchange needed).

Conv mapping: shift-and-accumulate matmuls with M = (ky, cg) = 7*17 = 119
packed output rows. The ky-summation is deferred: partial planes are stored
to a y-padded SBUF tensor with a per-ky y-shift via ONE DMA whose hand-built
access pattern strides both partitions (+17) and bytes (-row) per ky group.
The scores matmul contracts over (ky, cg) with a ky-replicated k_c as the
stationary operand, which completes the convolution for free.

Attention: scores computed transposed S^T[sk, sq] so softmax-exp output E^T
feeds the value matmul directly: rec^T[d|Z, sq] = [v|1]^T E^T. 1/Z via DVE
reciprocal on a transpose-gathered [16,64] tile + K=1 broadcast matmul +
DVE multiply.

Dtypes: everything on the PE in bf16 except the zb broadcast (bf16) and
f32 PSUM accumulation; E in bf16 halves the value-matmul cycles.

DMA queues: xq preload on the scalar queue, out stores on gpsimd, the rest
(consts, plane shifts, conv scatters, z transposes) on sync — avoids
head-of-line blocking on a single queue (each DMA issue costs ~0.6us).
"""

import functools
import numpy as np
import ml_dtypes

import bass_rust
import concourse.bass as bass
import concourse.tile as tile
import concourse.mybir as mybir
from concourse import bacc
from concourse.bass_utils import run_bass_kernel_spmd

F32 = mybir.dt.float32
F32R = mybir.dt.float32r
BF16 = mybir.dt.bfloat16

B = 4
NH = 4
D = 64            # head dim
CG = 17           # squeezed channels
K7 = 7            # spatial kernel
HQ = 64           # q image h=w
HK = 32           # k image h=w
SQ = HQ * HQ      # 4096
SK = HK * HK      # 1024
MC = K7 * CG      # 119 conv output rows (ky, cg)
QP = HQ + 6       # 70: x-padded q row width
KP = HK + 6       # 38: x-padded k row width
SCALE = D ** -0.5

QCH = 8           # q spatial chunks (8 y-rows each)
KCH = 2           # k spatial chunks (16 y-rows each)
QROWS = HQ // QCH  # 8
KROWS = HK // KCH  # 16
NSLAB = SQ // 512  # 8 sq slabs per head
FSQ = (HQ + 6) * HQ   # 4480: y-padded pq free size
FSK = (HK + 6) * HK   # 1216: y-padded pk free size
BODYQ = 3 * HQ        # 192: body column offset in pq
BODYK = 3 * HK        # 96

AF = mybir.ActivationFunctionType


def _build_program():
    nc = bacc.Bacc()

    xq = nc.dram_tensor("xq", [256, SQ], BF16, kind="ExternalInput")
    xkv = nc.dram_tensor("xkv", [256, SK], BF16, kind="ExternalInput")
    wqT = nc.dram_tensor("wqT", [256, 256], BF16, kind="ExternalInput")
    wkT = nc.dram_tensor("wkT", [256, 256], BF16, kind="ExternalInput")
    wvT = nc.dram_tensor("wvT", [256, 128], BF16, kind="ExternalInput")
    wcq = nc.dram_tensor("wcq", [128, 7, MC], BF16, kind="ExternalInput")
    wcq2 = nc.dram_tensor("wcq2", [128, 7, MC], BF16, kind="ExternalInput")
    wck = nc.dram_tensor("wck", [128, 7, MC], BF16, kind="ExternalInput")
    wck2 = nc.dram_tensor("wck2", [128, 7, MC], BF16, kind="ExternalInput")
    repl = nc.dram_tensor("repl", [MC, MC], BF16, kind="ExternalInput")
    out = nc.dram_tensor("out", [128, SQ], BF16, kind="ExternalOutput")

    with tile.TileContext(nc) as tc:
        _emit(nc, tc, xq, xkv, wqT, wkT, wvT, wcq, wcq2, wck, wck2, repl, out)
    nc.compile()
    return nc


def _shift_ap(dst_tile, y0, w, fs, cnt):
    """AP over a flat DRAM stage writing [119, cnt] planes with a per-ky
    y-shift: plane g (rows g*17..) lands cnt cols at (y0+6-g)*w. Mixed
    row/column strides are only legal on the DRAM side, hence the bounce."""
    v = dst_tile[:, (y0 + 6) * w:(y0 + 6) * w + cnt].copy()
    v.ap = bass_rust.VecI64Pair([[CG * fs - w, K7], [fs, CG], [1, cnt]])
    return v


def _emit(nc, tc, xq, xkv, wqT, wkT, wvT, wcq, wcq2, wck, wck2, repl, out):
    import contextlib
    ctx = contextlib.ExitStack()
    with ctx:
        consts = ctx.enter_context(tc.tile_pool(name="consts", bufs=1))
        stg = ctx.enter_context(tc.tile_pool(name="stg", bufs=3))
        dstg = ctx.enter_context(tc.tile_pool(name="dstg", bufs=2, space="DRAM"))
        pqp = ctx.enter_context(tc.tile_pool(name="pqp", bufs=2))
        pkp = ctx.enter_context(tc.tile_pool(name="pkp", bufs=2))
        k2p = ctx.enter_context(tc.tile_pool(name="k2p", bufs=2))
        ep = ctx.enter_context(tc.tile_pool(name="ep", bufs=2))
        rp = ctx.enter_context(tc.tile_pool(name="rp", bufs=2))
        zp = ctx.enter_context(tc.tile_pool(name="zp", bufs=2))
        op = ctx.enter_context(tc.tile_pool(name="op", bufs=3))
        ps_mm = ctx.enter_context(tc.tile_pool(name="ps_mm", bufs=2, space="PSUM"))
        ps_sc = ctx.enter_context(tc.tile_pool(name="ps_sc", bufs=2, space="PSUM"))
        ps_rec = ctx.enter_context(tc.tile_pool(name="ps_rec", bufs=2, space="PSUM"))

        # ---- xq preload on the scalar DMA queue (16 DMAs -> 16 engines) ----
        xq_sb = consts.tile([128, 2, SQ], BF16)
        for ct in range(2):
            for cb in range(8):
                nc.scalar.dma_start(
                    xq_sb[:, ct, cb * 512:(cb + 1) * 512],
                    xq[ct * 128:(ct + 1) * 128, cb * 512:(cb + 1) * 512])

        # ---- constant loads (sync queue), in first-use order: the v/k
        # projections only need xkv + small weights, so those go first and
        # the PE starts within a few us ----
        wkT_sb = consts.tile([128, 2, 256], BF16)
        for ct in range(2):
            nc.sync.dma_start(wkT_sb[:, ct, :],
                              wkT[ct * 128:(ct + 1) * 128, :])
        xkv_sb = consts.tile([128, 2, SK], BF16)
        for ct in range(2):
            for cb in range(2):
                nc.sync.dma_start(
                    xkv_sb[:, ct, cb * 512:(cb + 1) * 512],
                    xkv[ct * 128:(ct + 1) * 128, cb * 512:(cb + 1) * 512])
        wvT_sb = consts.tile([128, 2, 128], BF16)
        nc.sync.dma_start(wvT_sb, wvT.rearrange("(t p) m -> p t m", t=2))
        wck_sb = consts.tile([128, 7, MC], BF16)
        wck2_sb = consts.tile([128, 7, MC], BF16)
        nc.sync.dma_start(wck_sb, wck[:])
        nc.sync.dma_start(wck2_sb, wck2[:])
        wqT_sb = consts.tile([128, 2, 256], BF16)
        nc.sync.dma_start(wqT_sb, wqT.rearrange("(t p) m -> p t m", t=2))
        repl_sb = consts.tile([MC, MC], BF16)
        nc.sync.dma_start(repl_sb, repl[:])
        wcq_sb = consts.tile([128, 7, MC], BF16)
        wcq2_sb = consts.tile([128, 7, MC], BF16)
        nc.sync.dma_start(wcq_sb, wcq[:])
        nc.sync.dma_start(wcq2_sb, wcq2[:])
        ones0 = consts.tile([1, 64], BF16)
        nc.vector.memset(ones0[:], 1.0)
        zeros_sb = consts.tile([CG, 192], BF16)
        nc.vector.memset(zeros_sb[:], 0.0)

        def zero_stage_garbage(ds, w, rows):
            """Zero the never-written body-edge regions of a shifted DRAM
            stage: plane g only covers body cols from (3-g)*w (low) and up
            to (rows+3-g+3)*w (high)."""
            body0, body1 = 3 * w, (3 + rows) * w
            for g in range(3):
                gw = (3 - g) * w
                nc.scalar.dma_start(
                    ds[g * CG:(g + 1) * CG, body0:body0 + gw],
                    zeros_sb[:, 0:gw])
            for g in range(4, 7):
                gw = (g - 3) * w
                nc.scalar.dma_start(
                    ds[g * CG:(g + 1) * CG, body1 - gw:body1],
                    zeros_sb[:, 0:gw])

        # q/k plane tensors: [r0,r1], [r1,r2], [r2,r3]; x-padded (64|32)x(70|38)
        qA = consts.tile([128, HQ * QP], BF16)
        qB = consts.tile([128, HQ * QP], BF16)
        qC = consts.tile([128, HQ * QP], BF16)
        kA = consts.tile([128, HK * KP], BF16)
        kB = consts.tile([128, HK * KP], BF16)
        kC = consts.tile([128, HK * KP], BF16)

        # x-border zeros (cols 0..2 and 67..69 of each padded row)
        for t in (qA, qB, qC):
            v = t[:].rearrange("p (r c) -> p r c", c=QP)
            nc.vector.memset(v[:, :, 0:3], 0.0)
            nc.vector.memset(v[:, :, QP - 3:QP], 0.0)
        for t in (kA, kB, kC):
            v = t[:].rearrange("p (r c) -> p r c", c=KP)
            nc.vector.memset(v[:, :, 0:3], 0.0)
            nc.vector.memset(v[:, :, KP - 3:KP], 0.0)

        # v^T projection: out[sk_blk, (h0 d | h1 d)]
        v_sb = consts.tile([128, 8, 130], BF16)
        nc.vector.memset(v_sb[:, :, 64:65], 1.0)
        nc.vector.memset(v_sb[:, :, 129:130], 1.0)

        def v_proj():
            for t in range(8):
                acc = ps_mm.tile([128, 512], F32, tag="mm", name="accv")
                for ct in range(2):
                    nc.tensor.matmul(
                        acc[:, 0:128],
                        xkv_sb[:, ct, t * 128:(t + 1) * 128],
                        wvT_sb[:, ct, :],
                        start=(ct == 0), stop=(ct == 1),
                    )
                nc.vector.tensor_copy(v_sb[:, t, 0:64], acc[:, 0:64])
                nc.vector.tensor_copy(v_sb[:, t, 65:129], acc[:, 64:128])

        # k projection -> kA/kB/kC padded planes (bf16). kB (head slots
        # r1,r2) is projected directly with the middle weight columns
        # instead of a partition-shift DMA.
        def k_proj():
            for ch in range(2):
                for mt, (full, w0) in enumerate(
                        ((kA, 0), (kC, 128), (kB, 64))):
                    acc = ps_mm.tile([128, 512], F32, tag="mm", name="acckp")
                    for ct in range(2):
                        nc.tensor.matmul(
                            acc[:],
                            wkT_sb[:, ct, w0:w0 + 128],
                            xkv_sb[:, ct, ch * 512:(ch + 1) * 512],
                            start=(ct == 0), stop=(ct == 1),
                        )
                    rows = 512 // HK  # 16
                    y0 = ch * rows
                    dstf = full[:].rearrange("p (r c) -> p r c", c=KP)
                    accv = acc[:].rearrange("p (r c) -> p r c", c=HK)
                    nc.vector.tensor_copy(dstf[:, y0:y0 + rows, 3:3 + HK], accv)

        # q projection chunk -> qA/qB/qC (bf16)
        def q_proj_chunk(ch):
            for mt, (full, w0) in enumerate(((qA, 0), (qC, 128), (qB, 64))):
                acc = ps_mm.tile([128, 512], F32, tag="mm", name="accqp")
                for ct in range(2):
                    nc.tensor.matmul(
                        acc[:],
                        wqT_sb[:, ct, w0:w0 + 128],
                        xq_sb[:, ct, ch * 512:(ch + 1) * 512],
                        start=(ct == 0), stop=(ct == 1),
                    )
                rows = 512 // HQ  # 8
                y0 = ch * rows
                dstf = full[:].rearrange("p (r c) -> p r c", c=QP)
                accv = acc[:].rearrange("p (r c) -> p r c", c=HQ)
                nc.vector.tensor_copy(dstf[:, y0:y0 + rows, 3:3 + HQ], accv)

        # ---- per-head stage state ----
        pq_t = [None, None]
        pk_t = [None, None]
        k2_t = [None, None]
        rec_sb_t = [None, None]

        dsk_t = [None, None]
        dsq_t = [None, None]

        def conv_k_chunk(h, ch):
            k128 = (kA, kB)[h]
            sl = slice(0, 64) if h == 0 else slice(64, 128)
            if ch == 0:
                dsk_t[h] = dstg.tile([MC, FSK], BF16, tag="dsk", name="dsk")
                zero_stage_garbage(dsk_t[h], HK, HK)
            acc = ps_mm.tile([MC, 512], F32, tag="mm", name="acck")
            y0 = ch * KROWS
            for kx in range(K7):
                r1 = k128[:].rearrange("p (r c) -> p r c", c=KP)[
                    :, y0:y0 + KROWS, kx:kx + HK]
                nc.tensor.matmul(acc[:], wck_sb[:, kx, :], r1,
                                 start=(kx == 0), stop=False)
            for kx in range(K7):
                r2 = kC[:].rearrange("p (r c) -> p r c", c=KP)[
                    sl, y0:y0 + KROWS, kx:kx + HK]
                nc.tensor.matmul(acc[:], wck2_sb[sl, kx, :], r2,
                                 start=False, stop=(kx == K7 - 1))
            st = stg.tile([MC, 512], BF16, tag="stg", name="stk")
            nc.vector.tensor_copy(st[:], acc[:])
            nc.sync.dma_start(_shift_ap(dsk_t[h], y0, HK, FSK, KROWS * HK),
                              st[:])

        def load_pk(h):
            pk = pkp.tile([MC, FSK], BF16, tag="pk", name="pk")
            pk_t[h] = pk
            for i in range(2):
                c0 = BODYK + i * 512
                nc.scalar.dma_start(pk[:, c0:c0 + 512],
                                    dsk_t[h][:, c0:c0 + 512])

        def k2_build(h):
            k2 = k2p.tile([MC, SK], BF16, tag="k2", name="k2")
            k2_t[h] = k2
            for ch in range(2):
                acc = ps_mm.tile([MC, 512], F32, tag="mm", name="acc2")
                nc.tensor.matmul(
                    acc[:], repl_sb[:],
                    pk_t[h][:, BODYK + ch * 512:BODYK + (ch + 1) * 512],
                    start=True, stop=True)
                nc.vector.tensor_copy(k2[:, ch * 512:(ch + 1) * 512], acc[:])

        def conv_q_chunk(h, ch):
            q128 = (qA, qB)[h]
            sl = slice(0, 64) if h == 0 else slice(64, 128)
            if ch == 0:
                dsq_t[h] = dstg.tile([MC, FSQ], BF16, tag="dsq", name="dsq")
                zero_stage_garbage(dsq_t[h], HQ, HQ)
            acc = ps_mm.tile([MC, 512], F32, tag="mm", name="accq")
            y0 = ch * QROWS
            for kx in range(K7):
                r1 = q128[:].rearrange("p (r c) -> p r c", c=QP)[
                    :, y0:y0 + QROWS, kx:kx + HQ]
                nc.tensor.matmul(acc[:], wcq_sb[:, kx, :], r1,
                                 start=(kx == 0), stop=False)
            for kx in range(K7):
                r2 = qC[:].rearrange("p (r c) -> p r c", c=QP)[
                    sl, y0:y0 + QROWS, kx:kx + HQ]
                nc.tensor.matmul(acc[:], wcq2_sb[sl, kx, :], r2,
                                 start=False, stop=(kx == K7 - 1))
            st = stg.tile([MC, 512], BF16, tag="stg", name="stq")
            nc.vector.tensor_copy(st[:], acc[:])
            nc.sync.dma_start(_shift_ap(dsq_t[h], y0, HQ, FSQ, QROWS * HQ),
                              st[:])

        def load_pq_piece(h, i):
            if i == 0:
                pq_t[h] = pqp.tile([MC, FSQ], BF16, tag="pq", name="pq")
            c0 = BODYQ + i * 512
            nc.scalar.dma_start(pq_t[h][:, c0:c0 + 512],
                                dsq_t[h][:, c0:c0 + 512])

        def slab(h, s):
            if s == 0:
                rec_sb_t[h] = rp.tile([65, SQ], F32, tag="recsb", name="recsb")
            e_sb = ep.tile([128, SQ], BF16, tag="e", name="esb")
            for qq in range(4):
                sc = ps_sc.tile([128, 1024], F32, tag="sc", name="sc")
                for bb in range(2):
                    blk = qq * 2 + bb
                    nc.tensor.matmul(
                        sc[:, bb * 512:(bb + 1) * 512],
                        k2_t[h][:, blk * 128:(blk + 1) * 128],
                        pq_t[h][:, BODYQ + s * 512:BODYQ + (s + 1) * 512],
                        start=True, stop=True,
                    )
                nc.scalar.activation(
                    e_sb[:, qq * 1024:(qq + 1) * 1024], sc[:],
                    AF.Exp, scale=SCALE)
            rec = ps_rec.tile([65, 512], F32, tag="rec", name="rec")
            for t in range(8):
                nc.tensor.matmul(
                    rec[:],
                    v_sb[:, t, h * 65:(h + 1) * 65],
                    e_sb[:, t * 512:(t + 1) * 512],
                    start=(t == 0), stop=(t == 7),
                )
            nc.vector.tensor_copy(
                rec_sb_t[h][:, s * 512:(s + 1) * 512], rec[:])

        def divide_pair(h, p):
            """1/Z for slabs 2p,2p+1: transpose Z to 16 partitions via a
            DRAM bounce (SBUF APs can't cross partitions mid-row), DVE
            reciprocal multi-lane, transpose back."""
            rec_sb = rec_sb_t[h]
            c0 = p * 1024
            zrb = zp.tile([1, 1024], BF16, tag="zrb", name="zrb")
            zd1 = dstg.tile([1, 1024], F32, tag="zd1", name="zd1")
            nc.gpsimd.dma_start(zd1[:], rec_sb[64:65, c0:c0 + 1024])
            zt = zp.tile([16, 64], F32, tag="zt", name="zt")
            nc.gpsimd.dma_start(
                zt[:], zd1[0:1, :].rearrange("p (a b) -> (p a) b", a=16))
            zi = zp.tile([16, 64], F32R, tag="zi", name="zi")
            with nc.allow_low_precision(
                    reason="1/Z feeds bf16 broadcast mm"):
                nc.vector.reciprocal(zi[:], zt[:])
            zib = zp.tile([16, 64], BF16, tag="zib", name="zib")
            nc.vector.tensor_copy(zib[:], zi[:])
            zd2 = dstg.tile([1, 1024], BF16, tag="zd2", name="zd2")
            nc.gpsimd.dma_start(
                zd2[0:1, :].rearrange("p (a b) -> (p a) b", a=16), zib[:])
            nc.gpsimd.dma_start(zrb[:], zd2[:])
            emit_outputs(h, p, zrb)

        def emit_outputs(h, p, zrb):
            rec_sb = rec_sb_t[h]
            for j in range(2):
                s = 2 * p + j
                zb = ps_mm.tile([64, 512], F32, tag="mm", name="zb")
                nc.tensor.matmul(zb[:], ones0[:],
                                 zrb[0:1, j * 512:(j + 1) * 512],
                                 start=True, stop=True)
                ot = op.tile([64, 512], BF16, tag="ot", name="ot")
                with nc.allow_low_precision(reason="bf16 output store"):
                    nc.vector.tensor_mul(
                        ot[:], rec_sb[0:64, s * 512:(s + 1) * 512], zb[:])
                nc.gpsimd.dma_start(
                    out[h * 64:(h + 1) * 64, s * 512:(s + 1) * 512], ot[:])

        def divide_tail():
            """Slabs (1,6),(1,7): ACT ln/exp on the idle scalar engine —
            both lns then one exp so only 2 act-table loads hit the tail."""
            rec_sb = rec_sb_t[1]
            zl = zp.tile([1, 1024], F32, tag="zl", name="zl")
            for j, s in enumerate((6, 7)):
                nc.scalar.activation(zl[0:1, j * 512:(j + 1) * 512],
                                     rec_sb[64:65, s * 512:(s + 1) * 512],
                                     AF.Ln)
            zrb = zp.tile([1, 1024], BF16, tag="zrb", name="zrb")
            nc.scalar.activation(zrb[:], zl[:], AF.Exp, scale=-1.0)
            emit_outputs(1, 3, zrb)

        # ---- schedule ----
        # Phase A: k path (no xq dependency) warms the PE while xq preloads.
        k_proj()
        for ch in range(KCH):
            conv_k_chunk(0, ch)
        for ch in range(KCH):
            conv_k_chunk(1, ch)
        v_proj()
        load_pk(0)
        load_pk(1)
        # Phase B: q projection (hides the pk stage-write/load latency
        # before k2 consumes it), conv(h0) with pq gathers interleaved.
        for ch in range(4):
            q_proj_chunk(ch)
        k2_build(0)
        k2_build(1)
        for ch in range(4, QCH):
            q_proj_chunk(ch)
        for ch in range(QCH):
            conv_q_chunk(0, ch)
            if ch >= 1:
                load_pq_piece(0, ch - 1)
        load_pq_piece(0, QCH - 1)
        # Phase C: slabs(h0) with conv(h1) interleaved, then slabs(h1);
        # 1/Z + output divides trail one slab behind their pair.
        for h in range(2):
            for s in range(NSLAB):
                slab(h, s)
                if h == 0:
                    conv_q_chunk(1, s)
                    if s >= 1:
                        load_pq_piece(1, s - 1)
                k = h * NSLAB + s          # global slab index
                if k >= 2 and k % 2 == 0:
                    p = (k - 2) // 2       # pairs 0..6; 7 handled at tail
                    divide_pair(p // 4, p % 4)
            if h == 0:
                load_pq_piece(1, QCH - 1)
        divide_tail()


@functools.lru_cache(maxsize=1)
def _get_program():
    return _build_program()


def _host_inputs(xq, xkv, wq, wk, wv, emb_q, emb_k):
    """Build the 8 per-core input maps."""
    xq = np.ascontiguousarray(xq, dtype=np.float32)
    xkv = np.ascontiguousarray(xkv, dtype=np.float32)

    def conv_w(emb):
        # emb [cg, d, dnk, ky, kx] -> rows (dnk, d), cols (kx, ky, cg)
        arr = np.transpose(np.asarray(emb, np.float32), (2, 1, 4, 3, 0))
        w128 = np.ascontiguousarray(
            arr[0:2].reshape(128, 7, MC).astype(ml_dtypes.bfloat16))
        w64 = arr[2].reshape(64, 7, MC)
        w64d = np.ascontiguousarray(
            np.concatenate([w64, w64], axis=0).astype(ml_dtypes.bfloat16))
        return w128, w64d

    wcq_, wcq2_ = conv_w(emb_q)
    wck_, wck2_ = conv_w(emb_k)
    repl_ = np.ascontiguousarray(
        np.tile(np.eye(CG, dtype=np.float32), (K7, K7)).astype(
            ml_dtypes.bfloat16))

    wq = np.asarray(wq, np.float32)
    wk = np.asarray(wk, np.float32)
    wv = np.asarray(wv, np.float32)

    in_maps = []
    for core in range(8):
        b, p = divmod(core, 2)
        wqT_ = np.zeros((256, 256), np.float32)
        wkT_ = np.zeros((256, 256), np.float32)
        for j in range(4):
            head = 2 * p + j - 1
            if 0 <= head < NH:
                wqT_[:, j * 64:(j + 1) * 64] = wq[head * 64:(head + 1) * 64, :].T
                wkT_[:, j * 64:(j + 1) * 64] = wk[head * 64:(head + 1) * 64, :].T
        wvT_ = np.ascontiguousarray(wv[p * 128:(p + 1) * 128, :].T)
        in_maps.append(dict(
            xq=np.ascontiguousarray(
                xq[b].reshape(256, SQ).astype(ml_dtypes.bfloat16)),
            xkv=np.ascontiguousarray(
                xkv[b].reshape(256, SK).astype(ml_dtypes.bfloat16)),
            wqT=np.ascontiguousarray(wqT_.astype(ml_dtypes.bfloat16)),
            wkT=np.ascontiguousarray(wkT_.astype(ml_dtypes.bfloat16)),
            wvT=np.ascontiguousarray(wvT_.astype(ml_dtypes.bfloat16)),
            wcq=wcq_, wcq2=wcq2_, wck=wck_, wck2=wck2_,
            repl=repl_,
        ))
    return in_maps


def _run(inputs, **kw):
    nc = _get_program()
    in_maps = _host_inputs(**inputs)
    res = run_bass_kernel_spmd(nc, in_maps, core_ids=list(range(8)), **kw)
    outp = np.empty((B, 256, HQ, HQ), np.float32)
    for core in range(8):
        b, p = divmod(core, 2)
        outp[b, p * 128:(p + 1) * 128] = \
            res.results[core]["out"].reshape(128, HQ, HQ).astype(np.float32)
    return outp, res


def kernel(xq, xkv, wq, wk, wv, emb_q, emb_k):
    outp, _ = _run(dict(xq=xq, xkv=xkv, wq=wq, wk=wk, wv=wv,
                        emb_q=emb_q, emb_k=emb_k))
    return outp
